# revision 18
# baseline (speedup 1.0000x reference)
"""MiMo-V2 MoE gate routing kernel for 8 Trainium2 NeuronCores.

Problem: hidden_states [4,4096,4096] f32 -> gating GEMM vs 256 experts ->
sigmoid -> grouped top-k routing (8 groups, group score = sum of top-2,
keep top-4 groups, top-8 experts overall) -> normalized weights * 2.5.

Sharding: token-parallel. 16384 tokens / 8 cores = 2048 tokens per core.
Gate weight [256,4096] and bias are replicated. No cross-core comms.

Device layout (per core, default MODE="b3"):
  xh/xl [128, 16, 32, 128] bf16  x[p,tt,kc,t] = hi/lo bf16 plane of
                                 X[c*2048+tt*128+t, kc*128+p]
  wt    [128, 32, 512]     bf16  wt[p,kc,s*256+e] = {wh,wl}[s][e, kc*128+p]
                                 (flat 512-wide moving AP: 3D APs pay a
                                 ~50ns/MM segmentation bubble on HW)
  bias  [128, 256]         f32   bias[p,e]    = b[e]          (replicated)
  oidx  [128, 16, 8]       i32   oidx[t,tt,k] = topk_idx[tt*128+t, k]
  ow    [128, 16, 8]       f32   ow[t,tt,k]   = topk_weight[tt*128+t, k]

The GEMM puts tokens on PSUM partitions and experts on the free axis, so
all routing reductions are free-axis DVE ops; top-8 selection uses the
hardware InstMax / InstMaxIndex sort units.
"""

from contextlib import ExitStack

import numpy as np

import concourse.bacc as bacc
import concourse.mybir as mybir
import concourse.tile as tile
from concourse.bass_utils import run_bass_kernel_spmd

P = 128          # partitions
H = 4096         # hidden
E = 256          # experts
KC = H // P      # 32 contraction chunks
NCORES = 8
T = 16384        # total tokens
TPC = T // NCORES  # 2048 tokens per core
NT = TPC // P    # 16 token tiles per core
N_GROUP = 8
GSIZE = E // N_GROUP  # 32
TOPK_GROUP = 4
TOP_K = 8
ROUTED_SCALE = 2.5
NEG_BIG = 1.0e30

# GEMM mode:
#   "fp32" -- plain fp32 matmul (4 cyc/row, exact)
#   "r1"   -- float32r single pass (1 cyc/row, 12-bit operands: inexact)
#   "r3"   -- float32r 3-pass split (3 cyc/row total, fp32-class accuracy):
#             logits = xh@wh + xh@wl + xl@wh, where wh = trunc12(W),
#             wl = W - wh (host-split, 12-bit values pass through the PE's
#             fp32r rounding unchanged), xh = fp32r-cast(x) on device (the
#             same rounding the PE applies), xl = x - xh. The dropped term
#             xl@wl is ~2^-24 relative -- below fp32 accumulation noise.
#   "b3"   -- bf16 3-pass split, host-split both operands:
#             x = xh16 + xl16, w = wh16 + wl16 (bf16 RNE pairs). bf16
#             products are exact in the fp32 PSUM accumulate, so the only
#             error is the dropped xl@wl term (~2^-18 relative, ~1e-5 on
#             logits). Pass 1+2 fuse to one N=512 matmul per k-chunk
#             (moving = [wh|wl]); pass 3 is xl@wh at N=256. Unlike fp32r,
#             bf16 matmuls lower to LDWEIGHTS+MATMUL pairs the PE can
#             overlap, and x ships as two bf16 planes (same total bytes),
#             with no device-side DVE split at all.
MODE = "b3"
# b3 only: split pass 1+2 into same-stationary N=256 pairs and strip the
# second (redundant) InstLdweights of each pair. MEASURED WRONG on HW
# (31% idx match): every InstMatmult consumes the buffer of its own
# preceding LDW (dual-buffer flip), so the reload is mandatory. Stripping
# also saved ~0 time -- the LDW instructions are already hidden; the
# ~27ns/MM residual is dispatch/pipeline cost. Keep False.
LDW_SKIP = False
MM_DT = mybir.dt.float32  # used by fp32/r1 modes
TRACE = False
X_BUFS = 4      # x-tile double-buffer depth (DMA prefetch window)
X_SPLIT = 4     # dma_starts per x tile (spreads one tile across queues)
W_SPLIT = 8     # dma_starts for the weight preload
PROBE_SAME_LHS = False  # timing probe: reuse one stationary for all matmuls

_CACHE = {}


def _build(mm_dt, reps=1, mode=None):
    mode = mode or MODE
    if mode == "r1":
        mm_dt = mybir.dt.float32r
    elif mode == "r3":
        mm_dt = mybir.dt.float32r
    f32 = mybir.dt.float32
    nc = bacc.Bacc(
        "TRN2", target_bir_lowering=False, debug=False, enable_asserts=False
    )
    if mode == "b3":
        bf16 = mybir.dt.bfloat16
        xh = nc.dram_tensor("xh", [P, NT, KC, P], bf16, kind="ExternalInput").ap()
        xl = nc.dram_tensor("xl", [P, NT, KC, P], bf16, kind="ExternalInput").ap()
        wt = nc.dram_tensor("wt", [P, KC, 2 * E], bf16, kind="ExternalInput").ap()
        bias = nc.dram_tensor("bias", [P, E], f32, kind="ExternalInput").ap()
        oidx = nc.dram_tensor("oidx", [P, NT, TOP_K], mybir.dt.int32,
                              kind="ExternalOutput").ap()
        ow = nc.dram_tensor("ow", [P, NT, TOP_K], f32, kind="ExternalOutput").ap()
        with tile.TileContext(nc) as tc, ExitStack() as ctx:
            if reps == 1:
                _body_b3(ctx, tc, xh, xl, wt, bias, oidx, ow)
            else:
                with tc.For_i(0, reps, 1):
                    with ExitStack() as ictx:
                        _body_b3(ictx, tc, xh, xl, wt, bias, oidx, ow)
        nc.compile()
        if LDW_SKIP:
            _strip_dup_ldw(nc)
        return nc
    if mode == "r3":
        # x tiles are DVE-processed (split), so keep them float32 and view
        # as float32r only at the matmul; weights go straight from DMA.
        xt = nc.dram_tensor("xt", [P, NT, KC, P], f32, kind="ExternalInput").ap()
        wt = nc.dram_tensor("wt", [P, KC, 2, E], mm_dt, kind="ExternalInput").ap()
    else:
        xt = nc.dram_tensor("xt", [P, NT, KC, P], mm_dt, kind="ExternalInput").ap()
        wt = nc.dram_tensor("wt", [P, KC, E], mm_dt, kind="ExternalInput").ap()
    bias = nc.dram_tensor("bias", [P, E], f32, kind="ExternalInput").ap()
    oidx = nc.dram_tensor("oidx", [P, NT, TOP_K], mybir.dt.int32,
                          kind="ExternalOutput").ap()
    ow = nc.dram_tensor("ow", [P, NT, TOP_K], f32, kind="ExternalOutput").ap()

    with tile.TileContext(nc) as tc, ExitStack() as ctx:
        if reps == 1:
            _body(ctx, tc, xt, wt, bias, oidx, ow, mm_dt, mode)
        else:
            with tc.For_i(0, reps, 1):
                with ExitStack() as ictx:
                    _body(ictx, tc, xt, wt, bias, oidx, ow, mm_dt, mode)
    nc.compile()
    return nc


def _strip_dup_ldw(nc):
    """Drop redundant PE weight loads after bacc compilation.

    bacc lowers every matmul to an InstLdweights + InstMatmult(ldweights=
    False) pair. Consecutive matmuls that share a stationary (our pass-1/2
    pairs) then reload identical weights; the PE executes LDW serially at
    ~27ns each. Removing an InstLdweights whose access pattern matches the
    previous one on the PE queue (with no semaphore waits/updates attached)
    is semantics-preserving: the PE weight registers still hold that data.
    Tracking resets at block boundaries and at any non-matmul PE op.
    """
    dropped = 0
    for fn in nc.m.functions:
        for b in fn.blocks:
            insts = list(b.instructions)
            keep = []
            last_key = None
            changed = False
            for i in insts:
                tn = type(i).__name__
                if getattr(i, "engine", None) == mybir.EngineType.PE:
                    if tn == "InstLdweights":
                        ap = i.ins[0]
                        key = (ap.memref, ap.offset, str(ap.ap), str(ap.dtype))
                        if key == last_key and i.sync_info is None:
                            dropped += 1
                            changed = True
                            continue
                        last_key = key
                    elif tn != "InstMatmult":
                        last_key = None
                keep.append(i)
            if changed:
                b.instructions = keep
    return dropped


def _body_b3(ctx, tc, xh, xl, wt, bias, oidx, ow):
    nc = tc.nc
    f32 = mybir.dt.float32
    bf16 = mybir.dt.bfloat16
    Alu = mybir.AluOpType

    wpool = ctx.enter_context(tc.tile_pool(name="wpool", bufs=1))
    cpool = ctx.enter_context(tc.tile_pool(name="cpool", bufs=1))
    xpool = ctx.enter_context(tc.tile_pool(name="xpool", bufs=X_BUFS))
    psa = ctx.enter_context(tc.tile_pool(name="psa", bufs=4, space="PSUM"))
    psb = ctx.enter_context(tc.tile_pool(name="psb", bufs=4, space="PSUM"))
    spool = ctx.enter_context(tc.tile_pool(name="spool", bufs=4))
    gpool = ctx.enter_context(tc.tile_pool(name="gpool", bufs=3))
    apool = ctx.enter_context(tc.tile_pool(name="apool", bufs=1))

    wsb = wpool.tile([P, KC, 2 * E], bf16)
    for ws in range(W_SPLIT):
        lo, hi = ws * KC // W_SPLIT, (ws + 1) * KC // W_SPLIT
        nc.sync.dma_start(wsb[:, lo:hi], wt[:, lo:hi])
    bsb = cpool.tile([P, E], f32)
    nc.sync.dma_start(bsb[:], bias)
    oi_acc = apool.tile([P, NT, TOP_K], mybir.dt.int32)
    owt_acc = apool.tile([P, NT, TOP_K], f32)

    for tt in range(NT):
        xhs = xpool.tile([P, KC, P], bf16, tag="xhs")
        xls = xpool.tile([P, KC, P], bf16, tag="xls")
        for xs in range(X_SPLIT):
            lo, hi = xs * KC // X_SPLIT, (xs + 1) * KC // X_SPLIT
            nc.sync.dma_start(xhs[:, lo:hi], xh[:, tt, lo:hi])
            nc.sync.dma_start(xls[:, lo:hi], xl[:, tt, lo:hi])

        # pass 1+2: psA[:, 0:E] += xh @ wh, psA[:, E:] += xh @ wl;
        # pass 3: psB += xl @ wh
        lga = psa.tile([P, 2 * E], f32, tag="lga")
        if LDW_SKIP:
            for kc in range(KC):
                nc.tensor.matmul(lga[:, 0:E], lhsT=xhs[:, kc],
                                 rhs=wsb[:, kc, 0:E],
                                 start=(kc == 0), stop=False)
                m2 = nc.tensor.matmul(lga[:, E:2 * E], lhsT=xhs[:, kc],
                                      rhs=wsb[:, kc, E:2 * E],
                                      start=(kc == 0), stop=(kc == KC - 1))
                m2.ldweights = False
        else:
            for kc in range(KC):
                nc.tensor.matmul(lga[:], lhsT=xhs[:, kc], rhs=wsb[:, kc],
                                 start=(kc == 0), stop=(kc == KC - 1))
        lgb = psb.tile([P, E], f32, tag="lgb")
        for kc in range(KC):
            nc.tensor.matmul(lgb[:], lhsT=xls[:, kc], rhs=wsb[:, kc, 0:E],
                             start=(kc == 0), stop=(kc == KC - 1))

        # lg = psA.hi + psA.lo + psB ; scores = sigmoid(lg) (+ bias == 0).
        # Only one PSUM operand is legal per DVE op, so copy then chain adds.
        sc = spool.tile([P, E], f32, tag="sc")
        nc.vector.tensor_copy(sc[:], lga[:, 0:E])
        nc.vector.tensor_tensor(sc[:], sc[:], lga[:, E:2 * E], Alu.add)
        nc.vector.tensor_tensor(sc[:], sc[:], lgb[:], Alu.add)
        sg = spool.tile([P, E], f32, tag="sg")
        nc.scalar.activation(sg[:], sc[:], mybir.ActivationFunctionType.Sigmoid)
        nc.vector.tensor_add(sg[:], sg[:], bsb[:])
        sc3 = sg[:].rearrange("p (g k) -> p g k", g=N_GROUP)

        # group scores: sum of top-2 within each group of 32
        gt = gpool.tile([P, N_GROUP, 8], f32, tag="gt")
        for g in range(N_GROUP):
            nc.vector.max(gt[:, g], sc3[:, g])
        gs = gpool.tile([P, N_GROUP], f32, tag="gs")
        nc.vector.tensor_tensor(gs[:], gt[:, :, 0], gt[:, :, 1], Alu.add)

        # top-4 groups: mask = gs >= (4th largest group score)
        gm = gpool.tile([P, 8], f32, tag="gm")
        nc.vector.max(gm[:], gs[:])
        mk = gpool.tile([P, N_GROUP], f32, tag="mk")
        nc.vector.tensor_scalar(
            mk[:], gs[:], gm[:, TOPK_GROUP - 1:TOPK_GROUP], None, Alu.is_ge
        )
        nc.vector.tensor_scalar(mk[:], mk[:], 1.0, NEG_BIG, Alu.subtract, Alu.mult)

        tmp = spool.tile([P, E], f32, tag="tmp")
        tmp3 = tmp[:].rearrange("p (g k) -> p g k", g=N_GROUP)
        for g in range(N_GROUP):
            nc.vector.tensor_scalar(
                tmp3[:, g], sc3[:, g], mk[:, g:g + 1], None, Alu.add
            )

        v8 = gpool.tile([P, TOP_K], f32, tag="v8")
        nc.vector.max(v8[:], tmp[:])
        i8 = gpool.tile([P, TOP_K], mybir.dt.uint32, tag="i8")
        nc.vector.max_index(i8[:], v8[:], tmp[:])

        den = gpool.tile([P, 1], f32, tag="den")
        nc.vector.tensor_reduce(den[:], v8[:], axis=mybir.AxisListType.X, op=Alu.add)
        nc.vector.tensor_scalar_add(den[:], den[:], 1e-20)
        rec = gpool.tile([P, 1], f32, tag="rec")
        nc.vector.reciprocal(rec[:], den[:])
        nc.vector.tensor_scalar_mul(rec[:], rec[:], ROUTED_SCALE)
        nc.vector.tensor_scalar(owt_acc[:, tt], v8[:], rec[:], None, Alu.mult)
        nc.vector.tensor_copy(oi_acc[:, tt], i8[:])

    nc.sync.dma_start(oidx, oi_acc[:])
    nc.sync.dma_start(ow, owt_acc[:])


def _body(ctx, tc, xt, wt, bias, oidx, ow, mm_dt, mode):
    nc = tc.nc
    f32 = mybir.dt.float32
    u32 = mybir.dt.uint32
    Alu = mybir.AluOpType
    r3 = mode == "r3"

    wpool = ctx.enter_context(tc.tile_pool(name="wpool", bufs=1))
    cpool = ctx.enter_context(tc.tile_pool(name="cpool", bufs=1))
    xpool = ctx.enter_context(tc.tile_pool(name="xpool", bufs=X_BUFS))
    pspool = ctx.enter_context(tc.tile_pool(name="pspool", bufs=4, space="PSUM"))
    spool = ctx.enter_context(tc.tile_pool(name="spool", bufs=3))
    gpool = ctx.enter_context(tc.tile_pool(name="gpool", bufs=3))
    apool = ctx.enter_context(tc.tile_pool(name="apool", bufs=1))
    if r3:
        xlpool = ctx.enter_context(tc.tile_pool(name="xlpool", bufs=2))

    wsb = wpool.tile([P, KC, 2, E] if r3 else [P, KC, E], mm_dt)
    for ws in range(W_SPLIT):
        lo, hi = ws * KC // W_SPLIT, (ws + 1) * KC // W_SPLIT
        nc.sync.dma_start(wsb[:, lo:hi], wt[:, lo:hi])
    bsb = cpool.tile([P, E], f32)
    nc.sync.dma_start(bsb[:], bias)
    oi_acc = apool.tile([P, NT, TOP_K], mybir.dt.int32)
    owt_acc = apool.tile([P, NT, TOP_K], f32)

    for tt in range(NT):
        xsb = xpool.tile([P, KC, P], f32 if r3 else mm_dt, tag="xsb")
        for xs in range(X_SPLIT):
            lo, hi = xs * KC // X_SPLIT, (xs + 1) * KC // X_SPLIT
            nc.sync.dma_start(xsb[:, lo:hi], xt[:, tt, lo:hi])

        if r3:
            # xh = fp32r-round(x) via dtype-converting copy (the same
            # rounding the PE applies), xl = x - xh (exact low bits).
            # Halved so the first half overlaps the DMA tail of the second.
            r = mybir.dt.float32r
            xh = xlpool.tile([P, KC, P], r, tag="xh")
            xl = xlpool.tile([P, KC, P], r, tag="xl")
            qk = KC // X_SPLIT
            for q in range(X_SPLIT):
                h = slice(q * qk, (q + 1) * qk)
                nc.vector.tensor_copy(xh[:, h], xsb[:, h])
                nc.vector.tensor_tensor(
                    xl[:, h], xsb[:, h], xh[:, h].bitcast(f32), Alu.subtract,
                )

        lg = pspool.tile([P, E], f32, tag="lg")
        if r3:
            # all xh passes first: the PE only needs xl ~9us into the
            # tile, so the subtract hides under the xh matmuls
            for kc in range(KC):
                nc.tensor.matmul(lg[:], lhsT=xh[:, kc], rhs=wsb[:, kc, 0],
                                 start=(kc == 0), stop=False)
                nc.tensor.matmul(lg[:], lhsT=xh[:, kc], rhs=wsb[:, kc, 1],
                                 start=False, stop=False)
            for kc in range(KC):
                nc.tensor.matmul(lg[:], lhsT=xl[:, kc], rhs=wsb[:, kc, 0],
                                 start=False, stop=(kc == KC - 1))
        else:
            for kc in range(KC):
                nc.tensor.matmul(
                    lg[:], lhsT=xsb[:, 0 if PROBE_SAME_LHS else kc],
                    rhs=wsb[:, kc],
                    start=(kc == 0), stop=(kc == KC - 1),
                )

        # scores = sigmoid(logits); s_choice = scores + bias (bias == 0 in
        # this problem, so scores == s_choice and weights come from sc).
        sc = spool.tile([P, E], f32, tag="sc")
        nc.scalar.activation(sc[:], lg[:], mybir.ActivationFunctionType.Sigmoid)
        nc.vector.tensor_add(sc[:], sc[:], bsb[:])
        sc3 = sc[:].rearrange("p (g k) -> p g k", g=N_GROUP)

        # group scores: sum of top-2 within each group of 32
        gt = gpool.tile([P, N_GROUP, 8], f32, tag="gt")
        for g in range(N_GROUP):
            nc.vector.max(gt[:, g], sc3[:, g])
        gs = gpool.tile([P, N_GROUP], f32, tag="gs")
        nc.vector.tensor_tensor(gs[:], gt[:, :, 0], gt[:, :, 1], Alu.add)

        # top-4 groups: mask = gs >= (4th largest group score)
        gm = gpool.tile([P, 8], f32, tag="gm")
        nc.vector.max(gm[:], gs[:])
        mk = gpool.tile([P, N_GROUP], f32, tag="mk")
        nc.vector.tensor_scalar(
            mk[:], gs[:], gm[:, TOPK_GROUP - 1:TOPK_GROUP], None, Alu.is_ge
        )
        # mk -> 0 for selected groups, -1e30 for unselected
        nc.vector.tensor_scalar(mk[:], mk[:], 1.0, NEG_BIG, Alu.subtract, Alu.mult)

        # masked scores, per group (tensor_scalar broadcasts [P,1] scalars)
        tmp = spool.tile([P, E], f32, tag="tmp")
        tmp3 = tmp[:].rearrange("p (g k) -> p g k", g=N_GROUP)
        for g in range(N_GROUP):
            nc.vector.tensor_scalar(
                tmp3[:, g], sc3[:, g], mk[:, g:g + 1], None, Alu.add
            )

        # top-8 experts (HW sort unit); ties resolve to lowest index like jax
        v8 = gpool.tile([P, TOP_K], f32, tag="v8")
        nc.vector.max(v8[:], tmp[:])
        i8 = gpool.tile([P, TOP_K], mybir.dt.uint32, tag="i8")
        nc.vector.max_index(i8[:], v8[:], tmp[:])

        # normalize: w = v8 * (2.5 / (sum(v8) + 1e-20))
        den = gpool.tile([P, 1], f32, tag="den")
        nc.vector.tensor_reduce(den[:], v8[:], axis=mybir.AxisListType.X, op=Alu.add)
        nc.vector.tensor_scalar_add(den[:], den[:], 1e-20)
        rec = gpool.tile([P, 1], f32, tag="rec")
        nc.vector.reciprocal(rec[:], den[:])
        nc.vector.tensor_scalar_mul(rec[:], rec[:], ROUTED_SCALE)
        nc.vector.tensor_scalar(owt_acc[:, tt], v8[:], rec[:], None, Alu.mult)
        nc.vector.tensor_copy(oi_acc[:, tt], i8[:])

    nc.sync.dma_start(oidx, oi_acc[:])
    nc.sync.dma_start(ow, owt_acc[:])


def _get_nc():
    key = (MODE, MM_DT, X_BUFS, X_SPLIT, W_SPLIT)
    if key not in _CACHE:
        _CACHE[key] = _build(MM_DT)
    return _CACHE[key]


def _trunc12(a):
    """truncate to 12 mantissa bits (same split the kernel uses for x)."""
    return (a.view(np.uint32) & 0xFFFFF000).view(np.float32)


def make_in_maps(hidden_states, weight, e_score_correction_bias):
    x = np.ascontiguousarray(hidden_states, dtype=np.float32).reshape(T, H)
    wtt = np.ascontiguousarray(
        np.asarray(weight, dtype=np.float32).reshape(E, KC, P).transpose(2, 1, 0)
    )
    bias = np.ascontiguousarray(
        np.broadcast_to(
            np.asarray(e_score_correction_bias, dtype=np.float32), (P, E)
        )
    )
    if MODE == "b3":
        import ml_dtypes

        bf = ml_dtypes.bfloat16
        xh16 = x.astype(bf)
        xl16 = (x - xh16.astype(np.float32)).astype(bf)

        def xmap(a):
            # [c, tt, t, kc, p] -> [c, p, tt, kc, t]
            return np.ascontiguousarray(
                a.reshape(NCORES, NT, P, KC, P).transpose(0, 4, 1, 3, 2)
            )

        xhs, xls = xmap(xh16), xmap(xl16)
        wh16 = wtt.astype(bf)
        wl16 = (wtt - wh16.astype(np.float32)).astype(bf)
        wstack = np.ascontiguousarray(
            np.stack([wh16, wl16], axis=2).reshape(P, KC, 2 * E))
        return [
            {"xh": xhs[c], "xl": xls[c], "wt": wstack, "bias": bias}
            for c in range(NCORES)
        ]
    # [c, tt, t, kc, p] -> [c, p, tt, kc, t]
    xts = np.ascontiguousarray(
        x.reshape(NCORES, NT, P, KC, P).transpose(0, 4, 1, 3, 2)
    )
    if MODE == "r3":
        wh = _trunc12(wtt)
        wl = wtt - wh
        wtt = np.ascontiguousarray(np.stack([wh, wl], axis=2))  # [P, KC, 2, E]
    return [
        {"xt": xts[c], "wt": wtt, "bias": bias} for c in range(NCORES)
    ]


def gather_outputs(out_maps):
    idx = np.stack([m["oidx"] for m in out_maps])   # [c, p, tt, k]
    w = np.stack([m["ow"] for m in out_maps])
    idx = idx.transpose(0, 2, 1, 3).reshape(T, TOP_K)
    w = w.transpose(0, 2, 1, 3).reshape(T, TOP_K)
    return np.ascontiguousarray(idx.astype(np.int32)), np.ascontiguousarray(w)


def kernel(hidden_states, weight, e_score_correction_bias):
    nc = _get_nc()
    in_maps = make_in_maps(hidden_states, weight, e_score_correction_bias)
    res = run_bass_kernel_spmd(
        nc, in_maps, core_ids=list(range(NCORES)), trace=TRACE
    )
    kernel.last_results = res
    return gather_outputs(res.results)



# revision 20
# speedup vs baseline: 1.0508x; 1.0508x over previous
"""MiMo-V2 MoE gate routing kernel for 8 Trainium2 NeuronCores.

Problem: hidden_states [4,4096,4096] f32 -> gating GEMM vs 256 experts ->
sigmoid -> grouped top-k routing (8 groups, group score = sum of top-2,
keep top-4 groups, top-8 experts overall) -> normalized weights * 2.5.

Sharding: token-parallel. 16384 tokens / 8 cores = 2048 tokens per core.
Gate weight [256,4096] and bias are replicated. No cross-core comms.

Device layout (per core, default MODE="b3"):
  xh/xl [128, 16, 32, 128] bf16  x[p,tt,kc,t] = hi/lo bf16 plane of
                                 X[c*2048+tt*128+t, kc*128+p]
  wt    [128, 32, 512]     bf16  wt[p,kc,s*256+e] = {wh,wl}[s][e, kc*128+p]
                                 (flat 512-wide moving AP: 3D APs pay a
                                 ~50ns/MM segmentation bubble on HW)
  bias  [128, 256]         f32   bias[p,e]    = b[e]          (replicated)
  oidx  [128, 16, 8]       i32   oidx[t,tt,k] = topk_idx[tt*128+t, k]
  ow    [128, 16, 8]       f32   ow[t,tt,k]   = topk_weight[tt*128+t, k]

The GEMM puts tokens on PSUM partitions and experts on the free axis, so
all routing reductions are free-axis DVE ops; top-8 selection uses the
hardware InstMax / InstMaxIndex sort units.
"""

from contextlib import ExitStack

import numpy as np

import concourse.bacc as bacc
import concourse.mybir as mybir
import concourse.tile as tile
from concourse.bass_utils import run_bass_kernel_spmd

P = 128          # partitions
H = 4096         # hidden
E = 256          # experts
KC = H // P      # 32 contraction chunks
NCORES = 8
T = 16384        # total tokens
TPC = T // NCORES  # 2048 tokens per core
NT = TPC // P    # 16 token tiles per core
N_GROUP = 8
GSIZE = E // N_GROUP  # 32
TOPK_GROUP = 4
TOP_K = 8
ROUTED_SCALE = 2.5
NEG_BIG = 1.0e30

# GEMM mode:
#   "fp32" -- plain fp32 matmul (4 cyc/row, exact)
#   "r1"   -- float32r single pass (1 cyc/row, 12-bit operands: inexact)
#   "r3"   -- float32r 3-pass split (3 cyc/row total, fp32-class accuracy):
#             logits = xh@wh + xh@wl + xl@wh, where wh = trunc12(W),
#             wl = W - wh (host-split, 12-bit values pass through the PE's
#             fp32r rounding unchanged), xh = fp32r-cast(x) on device (the
#             same rounding the PE applies), xl = x - xh. The dropped term
#             xl@wl is ~2^-24 relative -- below fp32 accumulation noise.
#   "b3"   -- bf16 3-pass split, host-split both operands:
#             x = xh16 + xl16, w = wh16 + wl16 (bf16 RNE pairs). bf16
#             products are exact in the fp32 PSUM accumulate, so the only
#             error is the dropped xl@wl term (~2^-18 relative, ~1e-5 on
#             logits). Pass 1+2 fuse to one N=512 matmul per k-chunk
#             (moving = [wh|wl]); pass 3 is xl@wh at N=256. Unlike fp32r,
#             bf16 matmuls lower to LDWEIGHTS+MATMUL pairs the PE can
#             overlap, and x ships as two bf16 planes (same total bytes),
#             with no device-side DVE split at all.
MODE = "b3"
# b3 only: split pass 1+2 into same-stationary N=256 pairs and strip the
# second (redundant) InstLdweights of each pair. MEASURED WRONG on HW
# (31% idx match): every InstMatmult consumes the buffer of its own
# preceding LDW (dual-buffer flip), so the reload is mandatory. Stripping
# also saved ~0 time -- the LDW instructions are already hidden; the
# ~27ns/MM residual is dispatch/pipeline cost. Keep False.
LDW_SKIP = False
MM_DT = mybir.dt.float32  # used by fp32/r1 modes
TRACE = False
X_BUFS = 3      # x-tile double-buffer depth (DMA prefetch window)
X_SPLIT = 4     # dma_starts per x tile (spreads one tile across queues)
W_SPLIT = 8     # dma_starts for the weight preload
PROBE_SAME_LHS = False  # timing probe: reuse one stationary for all matmuls

_CACHE = {}


def _build(mm_dt, reps=1, mode=None):
    mode = mode or MODE
    if mode == "r1":
        mm_dt = mybir.dt.float32r
    elif mode == "r3":
        mm_dt = mybir.dt.float32r
    f32 = mybir.dt.float32
    nc = bacc.Bacc(
        "TRN2", target_bir_lowering=False, debug=False, enable_asserts=False
    )
    if mode == "b3":
        bf16 = mybir.dt.bfloat16
        xh = nc.dram_tensor("xh", [P, NT, KC, P], bf16, kind="ExternalInput").ap()
        xl = nc.dram_tensor("xl", [P, NT, KC, P], bf16, kind="ExternalInput").ap()
        wt = nc.dram_tensor("wt", [P, KC, 2 * E], bf16, kind="ExternalInput").ap()
        bias = nc.dram_tensor("bias", [P, E], f32, kind="ExternalInput").ap()
        oidx = nc.dram_tensor("oidx", [P, NT, TOP_K], mybir.dt.int32,
                              kind="ExternalOutput").ap()
        ow = nc.dram_tensor("ow", [P, NT, TOP_K], f32, kind="ExternalOutput").ap()
        with tile.TileContext(nc) as tc, ExitStack() as ctx:
            if reps == 1:
                _body_b3(ctx, tc, xh, xl, wt, bias, oidx, ow)
            else:
                with tc.For_i(0, reps, 1):
                    with ExitStack() as ictx:
                        _body_b3(ictx, tc, xh, xl, wt, bias, oidx, ow)
        nc.compile()
        if LDW_SKIP:
            _strip_dup_ldw(nc)
        return nc
    if mode == "r3":
        # x tiles are DVE-processed (split), so keep them float32 and view
        # as float32r only at the matmul; weights go straight from DMA.
        xt = nc.dram_tensor("xt", [P, NT, KC, P], f32, kind="ExternalInput").ap()
        wt = nc.dram_tensor("wt", [P, KC, 2, E], mm_dt, kind="ExternalInput").ap()
    else:
        xt = nc.dram_tensor("xt", [P, NT, KC, P], mm_dt, kind="ExternalInput").ap()
        wt = nc.dram_tensor("wt", [P, KC, E], mm_dt, kind="ExternalInput").ap()
    bias = nc.dram_tensor("bias", [P, E], f32, kind="ExternalInput").ap()
    oidx = nc.dram_tensor("oidx", [P, NT, TOP_K], mybir.dt.int32,
                          kind="ExternalOutput").ap()
    ow = nc.dram_tensor("ow", [P, NT, TOP_K], f32, kind="ExternalOutput").ap()

    with tile.TileContext(nc) as tc, ExitStack() as ctx:
        if reps == 1:
            _body(ctx, tc, xt, wt, bias, oidx, ow, mm_dt, mode)
        else:
            with tc.For_i(0, reps, 1):
                with ExitStack() as ictx:
                    _body(ictx, tc, xt, wt, bias, oidx, ow, mm_dt, mode)
    nc.compile()
    return nc


def _strip_dup_ldw(nc):
    """Drop redundant PE weight loads after bacc compilation.

    bacc lowers every matmul to an InstLdweights + InstMatmult(ldweights=
    False) pair. Consecutive matmuls that share a stationary (our pass-1/2
    pairs) then reload identical weights; the PE executes LDW serially at
    ~27ns each. Removing an InstLdweights whose access pattern matches the
    previous one on the PE queue (with no semaphore waits/updates attached)
    is semantics-preserving: the PE weight registers still hold that data.
    Tracking resets at block boundaries and at any non-matmul PE op.
    """
    dropped = 0
    for fn in nc.m.functions:
        for b in fn.blocks:
            insts = list(b.instructions)
            keep = []
            last_key = None
            changed = False
            for i in insts:
                tn = type(i).__name__
                if getattr(i, "engine", None) == mybir.EngineType.PE:
                    if tn == "InstLdweights":
                        ap = i.ins[0]
                        key = (ap.memref, ap.offset, str(ap.ap), str(ap.dtype))
                        if key == last_key and i.sync_info is None:
                            dropped += 1
                            changed = True
                            continue
                        last_key = key
                    elif tn != "InstMatmult":
                        last_key = None
                keep.append(i)
            if changed:
                b.instructions = keep
    return dropped


def _body_b3(ctx, tc, xh, xl, wt, bias, oidx, ow):
    nc = tc.nc
    f32 = mybir.dt.float32
    bf16 = mybir.dt.bfloat16
    Alu = mybir.AluOpType

    wpool = ctx.enter_context(tc.tile_pool(name="wpool", bufs=1))
    cpool = ctx.enter_context(tc.tile_pool(name="cpool", bufs=1))
    xpool = ctx.enter_context(tc.tile_pool(name="xpool", bufs=X_BUFS))
    psa = ctx.enter_context(tc.tile_pool(name="psa", bufs=3, space="PSUM"))
    psb = ctx.enter_context(tc.tile_pool(name="psb", bufs=3, space="PSUM"))
    spool = ctx.enter_context(tc.tile_pool(name="spool", bufs=4))
    gpool = ctx.enter_context(tc.tile_pool(name="gpool", bufs=3))
    apool = ctx.enter_context(tc.tile_pool(name="apool", bufs=1))

    wsb = wpool.tile([P, KC, 2 * E], bf16)
    for ws in range(W_SPLIT):
        lo, hi = ws * KC // W_SPLIT, (ws + 1) * KC // W_SPLIT
        nc.sync.dma_start(wsb[:, lo:hi], wt[:, lo:hi])
    bsb = cpool.tile([P, E], f32)
    nc.sync.dma_start(bsb[:], bias)
    oi_acc = apool.tile([P, NT, TOP_K], mybir.dt.int32)
    owt_acc = apool.tile([P, NT, TOP_K], f32)

    for tt in range(NT):
        xhs = xpool.tile([P, KC, P], bf16, tag="xhs")
        xls = xpool.tile([P, KC, P], bf16, tag="xls")
        for xs in range(X_SPLIT):
            lo, hi = xs * KC // X_SPLIT, (xs + 1) * KC // X_SPLIT
            nc.sync.dma_start(xhs[:, lo:hi], xh[:, tt, lo:hi])
            nc.sync.dma_start(xls[:, lo:hi], xl[:, tt, lo:hi])

        # pass 1+2: psA[:, 0:E] += xh @ wh, psA[:, E:] += xh @ wl;
        # pass 3: psB += xl @ wh
        lga = psa.tile([P, 2 * E], f32, tag="lga")
        if LDW_SKIP:
            for kc in range(KC):
                nc.tensor.matmul(lga[:, 0:E], lhsT=xhs[:, kc],
                                 rhs=wsb[:, kc, 0:E],
                                 start=(kc == 0), stop=False)
                m2 = nc.tensor.matmul(lga[:, E:2 * E], lhsT=xhs[:, kc],
                                      rhs=wsb[:, kc, E:2 * E],
                                      start=(kc == 0), stop=(kc == KC - 1))
                m2.ldweights = False
        else:
            for kc in range(KC):
                nc.tensor.matmul(lga[:], lhsT=xhs[:, kc], rhs=wsb[:, kc],
                                 start=(kc == 0), stop=(kc == KC - 1))
        lgb = psb.tile([P, E], f32, tag="lgb")
        for kc in range(KC):
            nc.tensor.matmul(lgb[:], lhsT=xls[:, kc], rhs=wsb[:, kc, 0:E],
                             start=(kc == 0), stop=(kc == KC - 1))

        # lg = psA.hi + psA.lo + psB ; scores = sigmoid(lg) (+ bias == 0).
        # Only one PSUM operand is legal per DVE op, so copy then chain adds.
        sc = spool.tile([P, E], f32, tag="sc")
        nc.vector.tensor_copy(sc[:], lga[:, 0:E])
        nc.vector.tensor_tensor(sc[:], sc[:], lga[:, E:2 * E], Alu.add)
        nc.vector.tensor_tensor(sc[:], sc[:], lgb[:], Alu.add)
        sg = spool.tile([P, E], f32, tag="sg")
        nc.scalar.activation(sg[:], sc[:], mybir.ActivationFunctionType.Sigmoid)
        nc.vector.tensor_add(sg[:], sg[:], bsb[:])
        sc3 = sg[:].rearrange("p (g k) -> p g k", g=N_GROUP)

        # group scores: sum of top-2 within each group of 32
        gt = gpool.tile([P, N_GROUP, 8], f32, tag="gt")
        for g in range(N_GROUP):
            nc.vector.max(gt[:, g], sc3[:, g])
        gs = gpool.tile([P, N_GROUP], f32, tag="gs")
        nc.vector.tensor_tensor(gs[:], gt[:, :, 0], gt[:, :, 1], Alu.add)

        # top-4 groups: mask = gs >= (4th largest group score)
        gm = gpool.tile([P, 8], f32, tag="gm")
        nc.vector.max(gm[:], gs[:])
        mk = gpool.tile([P, N_GROUP], f32, tag="mk")
        nc.vector.tensor_scalar(
            mk[:], gs[:], gm[:, TOPK_GROUP - 1:TOPK_GROUP], None, Alu.is_ge
        )
        nc.vector.tensor_scalar(mk[:], mk[:], 1.0, NEG_BIG, Alu.subtract, Alu.mult)

        tmp = spool.tile([P, E], f32, tag="tmp")
        tmp3 = tmp[:].rearrange("p (g k) -> p g k", g=N_GROUP)
        for g in range(N_GROUP):
            nc.vector.tensor_scalar(
                tmp3[:, g], sc3[:, g], mk[:, g:g + 1], None, Alu.add
            )

        v8 = gpool.tile([P, TOP_K], f32, tag="v8")
        nc.vector.max(v8[:], tmp[:])
        i8 = gpool.tile([P, TOP_K], mybir.dt.uint32, tag="i8")
        nc.vector.max_index(i8[:], v8[:], tmp[:])

        den = gpool.tile([P, 1], f32, tag="den")
        nc.vector.tensor_reduce(den[:], v8[:], axis=mybir.AxisListType.X, op=Alu.add)
        nc.vector.tensor_scalar_add(den[:], den[:], 1e-20)
        rec = gpool.tile([P, 1], f32, tag="rec")
        nc.vector.reciprocal(rec[:], den[:])
        nc.vector.tensor_scalar_mul(rec[:], rec[:], ROUTED_SCALE)
        nc.vector.tensor_scalar(owt_acc[:, tt], v8[:], rec[:], None, Alu.mult)
        nc.vector.tensor_copy(oi_acc[:, tt], i8[:])

    nc.sync.dma_start(oidx, oi_acc[:])
    nc.sync.dma_start(ow, owt_acc[:])


def _body(ctx, tc, xt, wt, bias, oidx, ow, mm_dt, mode):
    nc = tc.nc
    f32 = mybir.dt.float32
    u32 = mybir.dt.uint32
    Alu = mybir.AluOpType
    r3 = mode == "r3"

    wpool = ctx.enter_context(tc.tile_pool(name="wpool", bufs=1))
    cpool = ctx.enter_context(tc.tile_pool(name="cpool", bufs=1))
    xpool = ctx.enter_context(tc.tile_pool(name="xpool", bufs=X_BUFS))
    pspool = ctx.enter_context(tc.tile_pool(name="pspool", bufs=4, space="PSUM"))
    spool = ctx.enter_context(tc.tile_pool(name="spool", bufs=3))
    gpool = ctx.enter_context(tc.tile_pool(name="gpool", bufs=3))
    apool = ctx.enter_context(tc.tile_pool(name="apool", bufs=1))
    if r3:
        xlpool = ctx.enter_context(tc.tile_pool(name="xlpool", bufs=2))

    wsb = wpool.tile([P, KC, 2, E] if r3 else [P, KC, E], mm_dt)
    for ws in range(W_SPLIT):
        lo, hi = ws * KC // W_SPLIT, (ws + 1) * KC // W_SPLIT
        nc.sync.dma_start(wsb[:, lo:hi], wt[:, lo:hi])
    bsb = cpool.tile([P, E], f32)
    nc.sync.dma_start(bsb[:], bias)
    oi_acc = apool.tile([P, NT, TOP_K], mybir.dt.int32)
    owt_acc = apool.tile([P, NT, TOP_K], f32)

    for tt in range(NT):
        xsb = xpool.tile([P, KC, P], f32 if r3 else mm_dt, tag="xsb")
        for xs in range(X_SPLIT):
            lo, hi = xs * KC // X_SPLIT, (xs + 1) * KC // X_SPLIT
            nc.sync.dma_start(xsb[:, lo:hi], xt[:, tt, lo:hi])

        if r3:
            # xh = fp32r-round(x) via dtype-converting copy (the same
            # rounding the PE applies), xl = x - xh (exact low bits).
            # Halved so the first half overlaps the DMA tail of the second.
            r = mybir.dt.float32r
            xh = xlpool.tile([P, KC, P], r, tag="xh")
            xl = xlpool.tile([P, KC, P], r, tag="xl")
            qk = KC // X_SPLIT
            for q in range(X_SPLIT):
                h = slice(q * qk, (q + 1) * qk)
                nc.vector.tensor_copy(xh[:, h], xsb[:, h])
                nc.vector.tensor_tensor(
                    xl[:, h], xsb[:, h], xh[:, h].bitcast(f32), Alu.subtract,
                )

        lg = pspool.tile([P, E], f32, tag="lg")
        if r3:
            # all xh passes first: the PE only needs xl ~9us into the
            # tile, so the subtract hides under the xh matmuls
            for kc in range(KC):
                nc.tensor.matmul(lg[:], lhsT=xh[:, kc], rhs=wsb[:, kc, 0],
                                 start=(kc == 0), stop=False)
                nc.tensor.matmul(lg[:], lhsT=xh[:, kc], rhs=wsb[:, kc, 1],
                                 start=False, stop=False)
            for kc in range(KC):
                nc.tensor.matmul(lg[:], lhsT=xl[:, kc], rhs=wsb[:, kc, 0],
                                 start=False, stop=(kc == KC - 1))
        else:
            for kc in range(KC):
                nc.tensor.matmul(
                    lg[:], lhsT=xsb[:, 0 if PROBE_SAME_LHS else kc],
                    rhs=wsb[:, kc],
                    start=(kc == 0), stop=(kc == KC - 1),
                )

        # scores = sigmoid(logits); s_choice = scores + bias (bias == 0 in
        # this problem, so scores == s_choice and weights come from sc).
        sc = spool.tile([P, E], f32, tag="sc")
        nc.scalar.activation(sc[:], lg[:], mybir.ActivationFunctionType.Sigmoid)
        nc.vector.tensor_add(sc[:], sc[:], bsb[:])
        sc3 = sc[:].rearrange("p (g k) -> p g k", g=N_GROUP)

        # group scores: sum of top-2 within each group of 32
        gt = gpool.tile([P, N_GROUP, 8], f32, tag="gt")
        for g in range(N_GROUP):
            nc.vector.max(gt[:, g], sc3[:, g])
        gs = gpool.tile([P, N_GROUP], f32, tag="gs")
        nc.vector.tensor_tensor(gs[:], gt[:, :, 0], gt[:, :, 1], Alu.add)

        # top-4 groups: mask = gs >= (4th largest group score)
        gm = gpool.tile([P, 8], f32, tag="gm")
        nc.vector.max(gm[:], gs[:])
        mk = gpool.tile([P, N_GROUP], f32, tag="mk")
        nc.vector.tensor_scalar(
            mk[:], gs[:], gm[:, TOPK_GROUP - 1:TOPK_GROUP], None, Alu.is_ge
        )
        # mk -> 0 for selected groups, -1e30 for unselected
        nc.vector.tensor_scalar(mk[:], mk[:], 1.0, NEG_BIG, Alu.subtract, Alu.mult)

        # masked scores, per group (tensor_scalar broadcasts [P,1] scalars)
        tmp = spool.tile([P, E], f32, tag="tmp")
        tmp3 = tmp[:].rearrange("p (g k) -> p g k", g=N_GROUP)
        for g in range(N_GROUP):
            nc.vector.tensor_scalar(
                tmp3[:, g], sc3[:, g], mk[:, g:g + 1], None, Alu.add
            )

        # top-8 experts (HW sort unit); ties resolve to lowest index like jax
        v8 = gpool.tile([P, TOP_K], f32, tag="v8")
        nc.vector.max(v8[:], tmp[:])
        i8 = gpool.tile([P, TOP_K], mybir.dt.uint32, tag="i8")
        nc.vector.max_index(i8[:], v8[:], tmp[:])

        # normalize: w = v8 * (2.5 / (sum(v8) + 1e-20))
        den = gpool.tile([P, 1], f32, tag="den")
        nc.vector.tensor_reduce(den[:], v8[:], axis=mybir.AxisListType.X, op=Alu.add)
        nc.vector.tensor_scalar_add(den[:], den[:], 1e-20)
        rec = gpool.tile([P, 1], f32, tag="rec")
        nc.vector.reciprocal(rec[:], den[:])
        nc.vector.tensor_scalar_mul(rec[:], rec[:], ROUTED_SCALE)
        nc.vector.tensor_scalar(owt_acc[:, tt], v8[:], rec[:], None, Alu.mult)
        nc.vector.tensor_copy(oi_acc[:, tt], i8[:])

    nc.sync.dma_start(oidx, oi_acc[:])
    nc.sync.dma_start(ow, owt_acc[:])


def _get_nc():
    key = (MODE, MM_DT, X_BUFS, X_SPLIT, W_SPLIT)
    if key not in _CACHE:
        _CACHE[key] = _build(MM_DT)
    return _CACHE[key]


def _trunc12(a):
    """truncate to 12 mantissa bits (same split the kernel uses for x)."""
    return (a.view(np.uint32) & 0xFFFFF000).view(np.float32)


def make_in_maps(hidden_states, weight, e_score_correction_bias):
    x = np.ascontiguousarray(hidden_states, dtype=np.float32).reshape(T, H)
    wtt = np.ascontiguousarray(
        np.asarray(weight, dtype=np.float32).reshape(E, KC, P).transpose(2, 1, 0)
    )
    bias = np.ascontiguousarray(
        np.broadcast_to(
            np.asarray(e_score_correction_bias, dtype=np.float32), (P, E)
        )
    )
    if MODE == "b3":
        import ml_dtypes

        bf = ml_dtypes.bfloat16
        xh16 = x.astype(bf)
        xl16 = (x - xh16.astype(np.float32)).astype(bf)

        def xmap(a):
            # [c, tt, t, kc, p] -> [c, p, tt, kc, t]
            return np.ascontiguousarray(
                a.reshape(NCORES, NT, P, KC, P).transpose(0, 4, 1, 3, 2)
            )

        xhs, xls = xmap(xh16), xmap(xl16)
        wh16 = wtt.astype(bf)
        wl16 = (wtt - wh16.astype(np.float32)).astype(bf)
        wstack = np.ascontiguousarray(
            np.stack([wh16, wl16], axis=2).reshape(P, KC, 2 * E))
        return [
            {"xh": xhs[c], "xl": xls[c], "wt": wstack, "bias": bias}
            for c in range(NCORES)
        ]
    # [c, tt, t, kc, p] -> [c, p, tt, kc, t]
    xts = np.ascontiguousarray(
        x.reshape(NCORES, NT, P, KC, P).transpose(0, 4, 1, 3, 2)
    )
    if MODE == "r3":
        wh = _trunc12(wtt)
        wl = wtt - wh
        wtt = np.ascontiguousarray(np.stack([wh, wl], axis=2))  # [P, KC, 2, E]
    return [
        {"xt": xts[c], "wt": wtt, "bias": bias} for c in range(NCORES)
    ]


def gather_outputs(out_maps):
    idx = np.stack([m["oidx"] for m in out_maps])   # [c, p, tt, k]
    w = np.stack([m["ow"] for m in out_maps])
    idx = idx.transpose(0, 2, 1, 3).reshape(T, TOP_K)
    w = w.transpose(0, 2, 1, 3).reshape(T, TOP_K)
    return np.ascontiguousarray(idx.astype(np.int32)), np.ascontiguousarray(w)


def kernel(hidden_states, weight, e_score_correction_bias):
    nc = _get_nc()
    in_maps = make_in_maps(hidden_states, weight, e_score_correction_bias)
    res = run_bass_kernel_spmd(
        nc, in_maps, core_ids=list(range(NCORES)), trace=TRACE
    )
    kernel.last_results = res
    return gather_outputs(res.results)

